# revision 28
# baseline (speedup 1.0000x reference)
"""Trainium2 Bass kernel for nn_EncoderLayer (GNN message passing, 2-relation GAT).

Sharding: nodes (and incoming-edge lists, partitioned by dst) sharded across 8
cores; small GAT/FFN weights replicated; gathered src features fetched from a
replicated projection table via indexed DMA (dma_gather).

Per-core device program (v3):
  Phase 0: fold weights; stage all gather indices in SBUF.
  Phase 1: BN1 (vector-side rsqrt poly+Newton) + z/el projection for ALL
           nodes; packed rows zpackB[2*node + rel] = 12 x (64 z | 4 el)
           bf16, one batched x DMA per 4-block super-block, one zpack DMA
           per block.  Gather descriptor-gen for the first windows is
           issued here via prepare_only (deps defer to the triggers).
  Phase 2 (per dst-window, fully fused): trigger gather (both rels, one
           call); er recomputed on-chip from the window's x rows and
           broadcast edge-wise via the transposed one-hot ST matmul;
           ex = exp(leaky(el+er)) written into msgb by scalar (broadcast
           over dh), multiplied by z in place on vector; segment-sum via
           one-hot S matmuls in PSUM; m = msgsum/denom; x2 = x + m1 + m2;
           BN2; then the FFN (BN2 apply + 2 matmul layers + residual)
           interleaved in the same window iteration.  gpsimd runs ONLY
           gather preps/triggers so descriptor-gen pipelines ahead.
"""

import sys

sys.path.insert(0, "/opt/trn_rl_repo")

import numpy as np
import ml_dtypes

import concourse.bass as bass
import concourse.bacc as bacc
import concourse.tile as tile
import concourse.mybir as mybir
from concourse.bass_utils import run_bass_kernel_spmd

F32 = mybir.dt.float32
BF16 = mybir.dt.bfloat16
I16 = mybir.dt.int16
AF = mybir.ActivationFunctionType
ALU = mybir.AluOpType
BF16NP = ml_dtypes.bfloat16

N, T, D, H, DH, DFF = 10000, 12, 64, 4, 16, 128
NCORES = 8
CHUNK = N // NCORES          # 1250
WIN = 128                    # dst-window size (nodes)
NW = (CHUNK + WIN - 1) // WIN  # 10 windows; last has 98 nodes
EPS = 1e-5
NEG_SLOPE = 0.2
ZROW = 896                   # padded zpack row (bf16 elems): 12*68 data + 80 pad
NBLK = (N + 127) // 128      # 79 phase-1 blocks (last = 16 nodes)
NPAIR = T // 2               # 6 paired (2-timestep) transposes per block
SUP = 4                      # phase-1 super-block (batched DMA + rsqrt math)
PREPD = 2                    # gather prepare_only lookahead depth (= zg bufs)

# rsqrt(v + EPS) = quadratic fit + one Newton step (vector engine only).
_BN1_RANGE = (0.55, 1.6)
_BN2_RANGE = (0.55, 3.2)


def _rsqrt_coeffs(lo, hi):
    v = np.linspace(lo, hi, 4001)
    c = np.polyfit(v, 1.0 / np.sqrt(v + EPS), 2)
    return [float(x) for x in c]


def _win_nodes(w):
    return min(WIN, CHUNK - w * WIN)


def _prep_core_rel(src, dst, lo, B):
    """Edge lists for one (core, relation): sorted by dst, windowed, padded
    to B blocks of 128 edges per window. Returns (src_flat[NW*B*128], S, ST)
    with S[w, e_in_block, blk*128 + dst_local] and its per-block transpose
    ST[w, dst_local, blk*128 + e_in_block]."""
    hi = lo + CHUNK
    sel = (dst >= lo) & (dst < hi)
    es = src[sel].astype(np.int64)
    ed = (dst[sel] - lo).astype(np.int64)
    order = np.argsort(ed, kind="stable")
    es, ed = es[order], ed[order]
    L = NW * B * 128
    src_arr = np.zeros(L, np.int64)
    S = np.zeros((NW, 128, B * 128), BF16NP)
    ST = np.zeros((NW, 128, B * 128), BF16NP)
    wstart = np.searchsorted(ed, np.arange(NW) * WIN)
    wend = np.searchsorted(ed, np.arange(1, NW + 1) * WIN)
    for w in range(NW):
        seg_src = es[wstart[w]:wend[w]]
        seg_dst = ed[wstart[w]:wend[w]] - w * WIN
        cnt = len(seg_src)
        assert cnt <= B * 128
        base = w * B * 128
        src_arr[base:base + cnt] = seg_src
        i = np.arange(cnt)
        S[w, i % 128, (i // 128) * 128 + seg_dst] = 1.0
        ST[w, seg_dst, (i // 128) * 128 + (i % 128)] = 1.0
    return src_arr, S, ST


def _max_blocks(src, dst):
    best = 0
    for m in range(NCORES):
        lo = m * CHUNK
        sel = (dst >= lo) & (dst < lo + CHUNK)
        ed = dst[sel] - lo
        cnt = np.bincount(ed // WIN, minlength=NW)
        best = max(best, int(np.max((cnt + 127) // 128)))
    return best


def _emit_rsqrt(nc, pool, vcol, P, ncols, coeffs, tag):
    """rs = rsqrt(vcol + EPS) via quadratic + 1 Newton step, on vector."""
    Q2, Q1, Q0 = coeffs
    rs = pool.tile([128, ncols], F32, tag=f"rs{tag}")
    nc.vector.tensor_scalar(rs[:P], vcol, Q2, Q1, ALU.mult, ALU.add)
    nc.vector.tensor_mul(rs[:P], rs[:P], vcol)
    nc.vector.tensor_scalar(rs[:P], rs[:P], Q0, None, ALU.add)
    vep = pool.tile([128, ncols], F32, tag=f"vep{tag}")
    nc.vector.tensor_scalar(vep[:P], vcol, EPS, None, ALU.add)
    t_ = pool.tile([128, ncols], F32, tag=f"tn{tag}")
    nc.vector.tensor_mul(t_[:P], rs[:P], rs[:P])
    nc.vector.tensor_mul(t_[:P], t_[:P], vep[:P])
    nc.vector.tensor_scalar(t_[:P], t_[:P], -0.5, 1.5, ALU.mult, ALU.add)
    nc.vector.tensor_mul(rs[:P], rs[:P], t_[:P])
    return rs


def _build_program(B, phases=3):
    nc = bacc.Bacc("TRN2", target_bir_lowering=False, debug=False,
                   num_devices=NCORES)
    BL = B * 128               # padded edges per (window, rel)
    BL2 = 2 * BL
    W16 = BL2 // 16            # idx cols per window (both rels)
    L16 = NW * W16
    RC1 = _rsqrt_coeffs(*_BN1_RANGE)
    RC2 = _rsqrt_coeffs(*_BN2_RANGE)

    # ---- DRAM tensors ----
    x_full = nc.dram_tensor("x_full", [N, T * D], F32, kind="ExternalInput")
    xc = nc.dram_tensor("xc", [CHUNK, T * D], F32, kind="ExternalInput")
    bn1_gb = nc.dram_tensor("bn1_gb", [N, 2], F32, kind="ExternalInput")
    bn1_gbc = nc.dram_tensor("bn1_gbc", [CHUNK, 2], F32, kind="ExternalInput")
    bn2_gb = nc.dram_tensor("bn2_gb", [CHUNK, 2], F32, kind="ExternalInput")
    w_in, al_in, ar_in, s_in, st_in = [], [], [], [], []
    for r in (1, 2):
        w_in.append(nc.dram_tensor(f"W{r}", [D, H * DH], F32, kind="ExternalInput"))
        al_in.append(nc.dram_tensor(f"al{r}t", [D, H * DH], F32, kind="ExternalInput"))
        ar_in.append(nc.dram_tensor(f"ar{r}t", [D, H * DH], F32, kind="ExternalInput"))
        s_in.append(nc.dram_tensor(f"S{r}", [NW, 128, BL], BF16, kind="ExternalInput"))
        st_in.append(nc.dram_tensor(f"ST{r}", [NW, 128, BL], mybir.dt.float8e4, kind="ExternalInput"))
    si_in = nc.dram_tensor("srcidx", [128, L16], I16, kind="ExternalInput")
    ffw1_in = nc.dram_tensor("ffw1", [D, DFF], F32, kind="ExternalInput")
    ffb1_in = nc.dram_tensor("ffb1", [DFF, 1], F32, kind="ExternalInput")
    ffw2_in = nc.dram_tensor("ffw2", [DFF, D], F32, kind="ExternalInput")
    ffb2_in = nc.dram_tensor("ffb2", [D, 1], F32, kind="ExternalInput")
    ident_in = nc.dram_tensor("ident", [128, 128], BF16, kind="ExternalInput")
    out_d = nc.dram_tensor("OUT", [CHUNK, T * D], F32, kind="ExternalOutput")

    # interleaved: row 2*node + rel
    zpackB = nc.dram_tensor("zpackB", [2 * N, ZROW], BF16, kind="Internal")

    with tile.TileContext(nc) as tc:
        with (
            tc.tile_pool(name="const", bufs=1) as cpool,
            tc.tile_pool(name="zg", bufs=PREPD) as zgp,
        ):
            # ---- Phase 0 ----
            ident = cpool.tile([128, 128], BF16)
            nc.sync.dma_start(ident[:], ident_in[:])
            wpair = cpool.tile([128, 2, 2, 68], BF16)
            nc.vector.memset(wpair[:], 0.0)
            wer = cpool.tile([128, 2, 2, H], BF16)
            nc.vector.memset(wer[:], 0.0)
            for r in range(2):
                wf = cpool.tile([D, H * DH], F32, tag="wf")
                nc.sync.dma_start(wf[:], w_in[r][:])
                for par in range(2):
                    nc.vector.tensor_copy(
                        wpair[par * D:(par + 1) * D, r, par, 0:64], wf[:])
                for which, t_in in (("al", al_in[r]), ("ar", ar_in[r])):
                    alt = cpool.tile([D, H * DH], F32, tag="alt")
                    nc.sync.dma_start(alt[:], t_in[:])
                    prod = cpool.tile([D, H * DH], F32, tag="prod")
                    nc.vector.tensor_mul(prod[:], wf[:], alt[:])
                    red = cpool.tile([D, H], F32, tag="red")
                    nc.vector.tensor_reduce(
                        red[:].unsqueeze(2),
                        prod[:].rearrange("p (h k) -> p h k", k=DH),
                        mybir.AxisListType.X, ALU.add)
                    for par in range(2):
                        if which == "al":
                            nc.vector.tensor_copy(
                                wpair[par * D:(par + 1) * D, r, par, 64:68],
                                red[:])
                        else:
                            nc.vector.tensor_copy(
                                wer[par * D:(par + 1) * D, par, r, :], red[:])
            ffw1 = cpool.tile([128, DFF], BF16)
            t1 = cpool.tile([D, DFF], F32, tag="t1")
            nc.sync.dma_start(t1[:], ffw1_in[:])
            nc.vector.tensor_copy(ffw1[0:D, :], t1[:])
            nc.sync.dma_start(ffw1[64:128, :], ffw1[0:64, :])
            ffw2 = cpool.tile([DFF, D], BF16)
            t2 = cpool.tile([DFF, D], F32, tag="t2")
            nc.sync.dma_start(t2[:], ffw2_in[:])
            nc.vector.tensor_copy(ffw2[:], t2[:])
            ffb1 = cpool.tile([DFF, 1], F32)
            nc.sync.dma_start(ffb1[:], ffb1_in[:])
            ffb2r = cpool.tile([128, 1], F32)
            nc.sync.dma_start(ffb2r[0:64, :], ffb2_in[:])
            nc.sync.dma_start(ffb2r[64:128, :], ffb2_in[:])
            si_all = cpool.tile([128, L16], I16)
            nc.sync.dma_start(si_all[:], si_in[:])

            # prepare_only gathers: descriptor-gen runs as soon as the
            # gpsimd queue reaches the prep (data deps defer to the
            # trigger), so the first windows' desc-gen overlaps phase 1.
            # Consumers are gated manually via zgsem (16 incs per gather).
            zgs = {}
            zgsem = nc.alloc_semaphore("zgsem")

            def prep_gather(w):
                zg = zgp.tile([128, 2 * B, ZROW], BF16, tag="zg")
                nc.gpsimd.dma_gather(
                    zg[:], zpackB[:],
                    si_all[:, w * W16:(w + 1) * W16],
                    BL2, BL2, ZROW, single_packet=False,
                    prepare_only=True, sem=zgsem)
                zgs[w] = zg

            # ---- Phase 1: BN1 + projections for all N nodes ----
            with (
                tc.tile_pool(name="p1x", bufs=2) as p1x,
                tc.tile_pool(name="p1z", bufs=3) as p1z,
                tc.tile_pool(name="p1h", bufs=3) as p1h,
                tc.tile_pool(name="p1s", bufs=3) as p1s,
                tc.tile_pool(name="p1tp", bufs=2, space="PSUM") as p1tp,
                tc.tile_pool(name="p1zp", bufs=1, space="PSUM") as p1zp,
            ):
                for w in range(min(PREPD, NW)):
                    prep_gather(w)

                def stage_a(sb):
                    # batched x/gb DMA + stats + rsqrt poly for one super
                    nsb = min(SUP, NBLK - sb)
                    n0 = sb * 128
                    nn = min(SUP * 128, N - n0)
                    full = (nn == nsb * 128)
                    xt4 = p1x.tile([128, SUP, T * D], F32, tag="xt4")
                    gbt = p1s.tile([128, SUP, 2], F32, tag="gbt")
                    if full:
                        nc.sync.dma_start(
                            xt4[:, 0:nsb, :],
                            x_full[n0:n0 + nn].rearrange(
                                "(j p) c -> p j c", p=128))
                        nc.sync.dma_start(
                            gbt[:, 0:nsb, :],
                            bn1_gb[n0:n0 + nn].rearrange(
                                "(j p) c -> p j c", p=128))
                    mvt = p1s.tile([128, SUP, 2], F32, tag="mvt")
                    for j in range(nsb):
                        nb = min(128, N - (sb + j) * 128)
                        if not full:
                            nc.sync.dma_start(
                                xt4[:nb, j, :],
                                x_full[(sb + j) * 128:(sb + j) * 128 + nb])
                            nc.sync.dma_start(
                                gbt[:nb, j, :],
                                bn1_gb[(sb + j) * 128:(sb + j) * 128 + nb])
                        st6 = p1s.tile([128, 2, 6], F32, tag="st6")
                        nc.vector.bn_stats(st6[:nb, 0, :], xt4[:nb, j, 0:384])
                        nc.vector.bn_stats(st6[:nb, 1, :],
                                           xt4[:nb, j, 384:768])
                        nc.vector.bn_aggr(mvt[:nb, j, :], st6[:nb])
                    rs = _emit_rsqrt(nc, p1s, mvt[:, 0:nsb, 1], 128, nsb,
                                     RC1, "p1")
                    ab = p1s.tile([128, SUP, 2], F32, tag="ab")
                    nc.vector.tensor_mul(ab[:, 0:nsb, 0], gbt[:, 0:nsb, 0],
                                         rs[:, 0:nsb])
                    nc.vector.tensor_mul(ab[:, 0:nsb, 1], ab[:, 0:nsb, 0],
                                         mvt[:, 0:nsb, 0])
                    nc.vector.tensor_sub(ab[:, 0:nsb, 1], gbt[:, 0:nsb, 1],
                                         ab[:, 0:nsb, 1])
                    return xt4, ab, nsb

                supers = list(range(0, NBLK, SUP))
                staged = {supers[0]: stage_a(supers[0])}
                for ksb, sb in enumerate(supers):
                    if ksb + 1 < len(supers):
                        staged[supers[ksb + 1]] = stage_a(supers[ksb + 1])
                    xt4, ab, nsb = staged.pop(sb)
                    for j in range(nsb):
                        blk = sb + j
                        nb = min(128, N - blk * 128)
                        h = p1h.tile([128, T * D], BF16, tag="h")
                        nc.scalar.activation(h[:nb], xt4[:nb, j, :],
                                             AF.Identity,
                                             bias=ab[:nb, j, 1:2],
                                             scale=ab[:nb, j, 0:1])
                        tp = p1tp.tile([128, NPAIR, 128], BF16, tag="tp")
                        for p in range(NPAIR):
                            nc.tensor.transpose(
                                tp[:, p, 0:nb], h[:nb, p * 128:(p + 1) * 128],
                                ident[:nb, :nb])
                        ht = p1h.tile([128, NPAIR, 128], BF16, tag="ht")
                        if blk % 2 == 0:
                            nc.scalar.activation(ht[:, :, 0:nb],
                                                 tp[:, :, 0:nb], AF.Copy)
                        else:
                            nc.vector.tensor_copy(ht[:, :, 0:nb],
                                                  tp[:, :, 0:nb])
                        # one PSUM bank (512 f32) per pair: matmul outputs
                        # must never cross a 2KB PSUM bank boundary
                        zp = p1zp.tile([128, NPAIR, 512], F32, tag="zp")
                        for p in range(NPAIR):
                            nc.tensor.matmul(
                                zp[0:nb, p, 0:272], ht[:, p, 0:nb],
                                wpair[:].rearrange("p a b c -> p (a b c)"),
                                start=True, stop=True)
                        zel2 = p1z.tile([128, 2, T * 68], BF16, tag="zel2")
                        for r in range(2):
                            src_ap = zp[:nb, :, 0:272].rearrange(
                                "p q (r par c) -> p q r par c",
                                r=2, c=68)[:, :, r, :, :]
                            dst_ap = zel2[:nb, r, :].rearrange(
                                "p (q par c) -> p q par c", q=NPAIR, c=68)
                            if (blk + r) % 2 == 0:
                                nc.scalar.activation(dst_ap, src_ap, AF.Copy)
                            else:
                                nc.vector.tensor_copy(dst_ap, src_ap)
                        nc.sync.dma_start(
                            zpackB[2 * blk * 128:2 * blk * 128 + 2 * nb,
                                   0:T * 68],
                            zel2[:nb].rearrange("p r c -> p (r c)"))

            # ---- Phase 2: fused gather/attention/segment-sum/BN2/FFN ----
            with (
                tc.tile_pool(name="x2p", bufs=2) as x2p,
                tc.tile_pool(name="abp", bufs=2) as abp,
                tc.tile_pool(name="sp", bufs=2) as spp,
                tc.tile_pool(name="msg", bufs=2) as msgp,
                tc.tile_pool(name="p2s", bufs=4) as p2s,
                tc.tile_pool(name="p2t", bufs=2) as p2t,
                tc.tile_pool(name="pp", bufs=1, space="PSUM") as pp,
            ):
                pre = {}

                def prefetch(w):
                    nw = _win_nodes(w)
                    xcw = p2t.tile([128, T * D], F32, tag="xcw")
                    nc.sync.dma_start(xcw[:nw], xc[w * WIN:w * WIN + nw])
                    gb = p2s.tile([128, 2], F32, tag="gbw", bufs=2)
                    nc.sync.dma_start(gb[:nw], bn1_gbc[w * WIN:w * WIN + nw])
                    gb2 = p2s.tile([128, 2], F32, tag="gb2", bufs=2)
                    nc.sync.dma_start(gb2[:nw], bn2_gb[w * WIN:w * WIN + nw])
                    ss = []
                    for r in range(2):
                        ssb = spp.tile([128, BL], BF16, tag=f"ssb{r}")
                        nc.sync.dma_start(ssb[:], s_in[r][w])
                        stb = spp.tile([128, BL], mybir.dt.float8e4,
                                       tag=f"stb{r}", bufs=1)
                        nc.sync.dma_start(stb[:], st_in[r][w])
                        ss.append((ssb, stb))
                    pre[w] = (xcw, gb, gb2, ss)

                erws = {}

                def er_chain(w):
                    # one window ahead: only needs the prefetched x rows
                    nw = _win_nodes(w)
                    xcw, gb, gb2, ss = pre[w]
                    st6 = p2s.tile([128, 2, 6], F32, tag="st6w")
                    nc.vector.bn_stats(st6[:nw, 0, :], xcw[:nw, 0:384])
                    nc.vector.bn_stats(st6[:nw, 1, :], xcw[:nw, 384:768])
                    mv = p2s.tile([128, 2], F32, tag="mvw")
                    nc.vector.bn_aggr(mv[:nw], st6[:nw])
                    rsw = _emit_rsqrt(nc, p2s, mv[:nw, 1:2], nw, 1, RC1, "w")
                    a1 = p2s.tile([128, 2], F32, tag="a1w")
                    nc.vector.tensor_mul(a1[:nw, 0:1], gb[:nw, 0:1], rsw[:nw])
                    nc.vector.tensor_mul(a1[:nw, 1:2], a1[:nw, 0:1],
                                         mv[:nw, 0:1])
                    nc.vector.tensor_sub(a1[:nw, 1:2], gb[:nw, 1:2],
                                         a1[:nw, 1:2])
                    hw_ = p2t.tile([128, T * D], BF16, tag="hw")
                    nc.scalar.activation(hw_[:nw], xcw[:nw], AF.Identity,
                                         bias=a1[:nw, 1:2], scale=a1[:nw, 0:1])
                    tpw = pp.tile([128, NPAIR, 128], BF16, tag="tpS")
                    for p in range(NPAIR):
                        nc.tensor.transpose(
                            tpw[:, p, 0:nw], hw_[:nw, p * 128:(p + 1) * 128],
                            ident[:nw, :nw])
                    htw = p2t.tile([128, NPAIR, 128], BF16, tag="htw")
                    nc.vector.tensor_copy(htw[:, :, 0:nw], tpw[:, :, 0:nw])
                    erft = pp.tile([128, 816], F32, tag="ffn", bufs=1,
                                   name="erft")
                    erps = erft[:, 0:96].rearrange("p (q c) -> p q c",
                                                   c=2 * 2 * H)
                    for p in range(NPAIR):
                        nc.tensor.matmul(
                            erps[0:nw, p, :], htw[:, p, 0:nw],
                            wer[:].rearrange("p a b c -> p (a b c)"),
                            start=True, stop=True)
                    erw = p2s.tile([128, 2, T, H], mybir.dt.float8e4,
                                   tag="erw", bufs=2)
                    if nw < 128:
                        nc.vector.memset(erw[:], 0.0)
                    nc.vector.tensor_copy(
                        erw[:nw].rearrange("p r (q par) h -> p q par r h",
                                           par=2),
                        erps[:nw].rearrange("p q (par r h) -> p q par r h",
                                            par=2, r=2))
                    erws[w] = erw

                prefetch(0)
                er_chain(0)
                for w in range(NW):
                    nw = _win_nodes(w)
                    if w == 0 or w + 1 < NW:
                        nc.gpsimd.trigger_dma(count=None)
                    if w + PREPD < NW:
                        prep_gather(w + PREPD)
                    if w + 1 < NW:
                        prefetch(w + 1)
                        er_chain(w + 1)
                    zg = zgs.pop(w)
                    xcw, gb, gb2, ss = pre.pop(w)
                    erw = erws.pop(w)
                    # gate this window's zg readers on the gather DMA
                    nc.vector.wait_ge(zgsem, 16 * (w + 1))
                    msum = []
                    for r in range(2):
                        ssb, stb = ss[r]
                        lk = p2s.tile([128, B, T * H], BF16, tag="lk")
                        HB = (B + 1) // 2
                        for half in range(2):
                            b0 = half * HB
                            b1 = min(b0 + HB, B)
                            if b0 >= b1:
                                continue
                            ebc = pp.tile([128, HB, T * H], F32, tag="ebc")
                            for b in range(b0, b1):
                                nc.tensor.matmul(
                                    ebc[:, b - b0, :],
                                    stb[:, b * 128:(b + 1) * 128],
                                    erw[:, r, :, :].rearrange(
                                        "p q h -> p (q h)"),
                                    start=True, stop=True)
                            el_ap = zg[:, r * B + b0:r * B + b1, 0:T * 68] \
                                .rearrange("p b (t c) -> p b t c",
                                           c=68)[:, :, :, 64:68]
                            nc.vector.tensor_add(
                                lk[:, b0:b1].rearrange(
                                    "p b (t h) -> p b t h", h=H),
                                el_ap,
                                ebc[:, 0:b1 - b0].rearrange(
                                    "p b (t h) -> p b t h", h=H))
                        nc.vector.scalar_tensor_tensor(
                            lk[:], lk[:], NEG_SLOPE, lk[:], ALU.mult, ALU.max)
                        msgb = msgp.tile([128, B, 816], BF16, tag="msg")
                        nc.scalar.activation(
                            msgb[:, :, 0:768].rearrange(
                                "p b (t h k) -> p b t h k", h=H, k=DH),
                            lk[:].rearrange("p b (t h) -> p b t h", h=H)
                            .unsqueeze(4).broadcast_to((128, B, T, H, DH)),
                            AF.Exp)
                        nc.scalar.activation(msgb[:, :, 768:816], lk[:],
                                             AF.Exp)
                        zap = zg[:, r * B:(r + 1) * B, 0:T * 68].rearrange(
                            "p b (t c) -> p b t c", c=68)[:, :, :, 0:64]
                        mz = msgb[:, :, 0:768].rearrange(
                            "p b (t c) -> p b t c", c=64)
                        nc.vector.tensor_mul(mz, mz, zap)
                        ms = pp.tile([128, 816], F32, tag="msum", bufs=2)
                        for b in range(B):
                            lhsT = ssb[:, b * 128:(b + 1) * 128]
                            nc.tensor.matmul(ms[:, 0:512], lhsT,
                                             msgb[:, b, 0:512],
                                             start=(b == 0), stop=(b == B - 1))
                            nc.tensor.matmul(ms[:, 512:816], lhsT,
                                             msgb[:, b, 512:816],
                                             start=(b == 0), stop=(b == B - 1))
                        msum.append(ms)
                    # epilogue: m = msgsum/denom; x2 = bf16(x + m1 + m2)
                    x2w = x2p.tile([128, T * D], BF16, tag="x2")
                    mtmp = p2t.tile([128, T * D], F32, tag="mtmp")
                    for r in range(2):
                        rec = p2s.tile([128, T * H], F32, tag="rec")
                        nc.vector.tensor_scalar_max(
                            rec[:nw], msum[r][:nw, 768:816], 1e-16)
                        nc.vector.reciprocal(rec[:nw], rec[:nw])
                        rb = rec[:nw].rearrange(
                            "p (t h) -> p t h", h=H).unsqueeze(3) \
                            .broadcast_to((nw, T, H, DH))
                        dst = (mtmp if r == 0 else x2w)
                        nc.vector.tensor_mul(
                            dst[:nw].rearrange(
                                "p (t h k) -> p t h k", h=H, k=DH),
                            msum[r][:nw, 0:768].rearrange(
                                "p (t h k) -> p t h k", h=H, k=DH), rb)
                    nc.vector.tensor_add(mtmp[:nw], mtmp[:nw], xcw[:nw])
                    nc.vector.tensor_add(x2w[:nw], x2w[:nw], mtmp[:nw])
                    # BN2 stats + a2/b2
                    st6b = p2s.tile([128, 2, 6], F32, tag="st6b")
                    nc.vector.bn_stats(st6b[:nw, 0, :], x2w[:nw, 0:384])
                    nc.vector.bn_stats(st6b[:nw, 1, :], x2w[:nw, 384:768])
                    mvb = p2s.tile([128, 2], F32, tag="mvb")
                    nc.vector.bn_aggr(mvb[:nw], st6b[:nw])
                    rs2 = _emit_rsqrt(nc, p2s, mvb[:nw, 1:2], nw, 1, RC2,
                                      "b2")
                    ab2 = abp.tile([128, 2], F32, tag="ab2")
                    nc.vector.tensor_mul(ab2[:nw, 0:1], gb2[:nw, 0:1],
                                         rs2[:nw])
                    nc.vector.tensor_mul(ab2[:nw, 1:2], ab2[:nw, 0:1],
                                         mvb[:nw, 0:1])
                    nc.vector.tensor_sub(ab2[:nw, 1:2], gb2[:nw, 1:2],
                                         ab2[:nw, 1:2])
                    if phases < 3:
                        xo = p2t.tile([128, T * D], F32, tag="mtmp")
                        nc.vector.tensor_copy(xo[:nw], x2w[:nw])
                        nc.sync.dma_start(out_d[w * WIN:w * WIN + nw],
                                          xo[:nw])
                        continue
                    # ---- FFN (interleaved): BN2 apply + 2 layers + res ----
                    h2 = p2t.tile([128, T * D], BF16, tag="hw")
                    nc.scalar.activation(h2[:nw], x2w[:nw], AF.Identity,
                                         bias=ab2[:nw, 1:2],
                                         scale=ab2[:nw, 0:1])
                    h2t = p2t.tile([64, T, 128], BF16, tag="h2t")
                    for half in range(2):
                        tp = pp.tile([64, NPAIR, 128], BF16, tag="tpS")
                        for j in range(NPAIR):
                            t = half * NPAIR + j
                            nc.tensor.transpose(
                                tp[:, j, 0:nw], h2[:nw, t * 64:(t + 1) * 64],
                                ident[:nw, :nw])
                        nc.vector.tensor_copy(
                            h2t[:, half * NPAIR:(half + 1) * NPAIR, 0:nw],
                            tp[:, :, 0:nw])
                    if nw < 128:
                        nc.vector.memset(h2t[:, :, nw:128], 0.0)
                    dd = pp.tile([128, T, 64], BF16, tag="tpS")
                    fft = p2t.tile([64, T, 128], BF16, tag="fft")
                    for k in range(3):
                        big = pp.tile([128, 816], F32, tag="ffn", bufs=1)
                        f1 = big[:, 0:512]
                        rhs = h2t[:, 4 * k:4 * k + 4, :]
                        nc.tensor.matmul(f1, ffw1[0:64, :],
                                         rhs.rearrange("p a b -> p (a b)"),
                                         start=True, stop=True)
                        g1 = p2t.tile([128, 512], BF16, tag="g1")
                        nc.scalar.activation(g1[:], f1, AF.Gelu, bias=ffb1[:])
                        for half in range(2):
                            f2 = big[0:64, 512:768]
                            nc.tensor.matmul(
                                f2, ffw2[:],
                                g1[:, half * 256:(half + 1) * 256],
                                start=True, stop=True)
                            nc.scalar.activation(
                                fft[:, 4 * k + 2 * half:
                                    4 * k + 2 * half + 2, :]
                                .rearrange("p a b -> p (a b)"),
                                f2, AF.Identity, bias=ffb2r[0:64, :])
                    for t in range(T):
                        nc.tensor.transpose(
                            dd[0:nw, t, :], fft[:, t, 0:nw],
                            ident[0:64, 0:64])
                    ot = p2t.tile([128, T * D], F32, tag="mtmp")
                    nc.vector.tensor_add(
                        ot[:nw], dd[:nw].rearrange("p a b -> p (a b)"),
                        x2w[:nw])
                    nc.sync.dma_start(out_d[w * WIN:w * WIN + nw], ot[:nw])

    nc.compile()
    return nc


_CACHE = {}
_PHASES = 3
_TRACE = False
_TRACE_DIR = None
_LAST_EXEC_NS = None


def _host_prep(inputs):
    x = np.asarray(inputs["x"], np.float32)
    xf = np.ascontiguousarray(x.reshape(N, T * D))
    B = 0
    for r in (1, 2):
        B = max(B, _max_blocks(np.asarray(inputs[f"src{r}"]),
                               np.asarray(inputs[f"dst{r}"])))

    bn1_gb = np.ascontiguousarray(
        np.stack([np.asarray(inputs["bn1_g"], np.float32),
                  np.asarray(inputs["bn1_b"], np.float32)], axis=1))
    bn2_gb_full = np.ascontiguousarray(
        np.stack([np.asarray(inputs["bn2_g"], np.float32),
                  np.asarray(inputs["bn2_b"], np.float32)], axis=1))
    common = {
        "x_full": xf,
        "bn1_gb": bn1_gb,
        "ffw1": np.ascontiguousarray(np.asarray(inputs["ff_w1"], np.float32)),
        "ffb1": np.ascontiguousarray(
            np.asarray(inputs["ff_b1"], np.float32).reshape(DFF, 1)),
        "ffw2": np.ascontiguousarray(np.asarray(inputs["ff_w2"], np.float32)),
        "ffb2": np.ascontiguousarray(
            np.asarray(inputs["ff_b2"], np.float32).reshape(D, 1)),
        "ident": np.eye(128, dtype=BF16NP),
    }
    for r in (1, 2):
        W = np.asarray(inputs[f"W{r}"], np.float32).reshape(D, H * DH)
        al = np.asarray(inputs[f"al{r}"], np.float32).reshape(-1)
        ar = np.asarray(inputs[f"ar{r}"], np.float32).reshape(-1)
        common[f"W{r}"] = np.ascontiguousarray(W)
        common[f"al{r}t"] = np.ascontiguousarray(np.tile(al[None, :], (D, 1)))
        common[f"ar{r}t"] = np.ascontiguousarray(np.tile(ar[None, :], (D, 1)))

    BL = B * 128
    in_maps = []
    for m in range(NCORES):
        lo = m * CHUNK
        im = dict(common)
        im["xc"] = np.ascontiguousarray(xf[lo:lo + CHUNK])
        im["bn1_gbc"] = np.ascontiguousarray(bn1_gb[lo:lo + CHUNK])
        im["bn2_gb"] = np.ascontiguousarray(bn2_gb_full[lo:lo + CHUNK])
        srcs = []
        for r in (1, 2):
            src_flat, S, ST = _prep_core_rel(
                np.asarray(inputs[f"src{r}"]), np.asarray(inputs[f"dst{r}"]),
                lo, B)
            im[f"S{r}"] = S
            im[f"ST{r}"] = ST.astype(ml_dtypes.float8_e4m3)
            srcs.append(2 * src_flat + (r - 1))  # interleaved zpackB rows
        idx = np.empty((128, NW * (2 * BL) // 16), np.int16)
        for w in range(NW):
            for r in range(2):
                seg = srcs[r][w * BL:(w + 1) * BL].astype(np.int16)
                col0 = (w * 2 + r) * (BL // 16)
                idx[:, col0:col0 + BL // 16] = np.tile(
                    seg.reshape(-1, 16).T, (8, 1))
        im["srcidx"] = np.ascontiguousarray(idx)
        in_maps.append(im)
    return B, in_maps


def kernel(**inputs):
    B, in_maps = _host_prep(inputs)
    key = (B, _PHASES)
    if key not in _CACHE:
        _CACHE[key] = _build_program(B, _PHASES)
    nc = _CACHE[key]
    global _LAST_EXEC_NS
    tmpdir = None
    if _TRACE and _TRACE_DIR:
        import os, shutil
        shutil.rmtree(_TRACE_DIR, ignore_errors=True)
        os.makedirs(_TRACE_DIR, exist_ok=True)
        tmpdir = _TRACE_DIR
    res = run_bass_kernel_spmd(nc, in_maps, core_ids=list(range(NCORES)),
                               trace=_TRACE, tmpdir=tmpdir)
    _LAST_EXEC_NS = res.exec_time_ns
    out = np.concatenate([res.results[m]["OUT"] for m in range(NCORES)],
                         axis=0)
    return out.reshape(N, T, D).astype(np.float32)


# revision 29
# speedup vs baseline: 1.1197x; 1.1197x over previous
"""Trainium2 Bass kernel for nn_EncoderLayer (GNN message passing, 2-relation GAT).

Sharding: nodes (and incoming-edge lists, partitioned by dst) sharded across 8
cores; small GAT/FFN weights replicated; gathered src features fetched from a
replicated projection table via indexed DMA (dma_gather).

Per-core device program (v3):
  Phase 0: fold weights; stage all gather indices in SBUF.
  Phase 1: BN1 (vector-side rsqrt poly+Newton) + z/el projection for ALL
           nodes; packed rows zpackB[2*node + rel] = 12 x (64 z | 4 el)
           bf16, one batched x DMA per 4-block super-block, one zpack DMA
           per block.  Gather descriptor-gen for the first windows is
           issued here via prepare_only (deps defer to the triggers).
  Phase 2 (per dst-window, fully fused): trigger gather (both rels, one
           call); er recomputed on-chip from the window's x rows and
           broadcast edge-wise via the transposed one-hot ST matmul;
           ex = exp(leaky(el+er)) written into msgb by scalar (broadcast
           over dh), multiplied by z in place on vector; segment-sum via
           one-hot S matmuls in PSUM; m = msgsum/denom; x2 = x + m1 + m2;
           BN2; then the FFN (BN2 apply + 2 matmul layers + residual)
           interleaved in the same window iteration.  gpsimd runs ONLY
           gather preps/triggers so descriptor-gen pipelines ahead.
"""

import sys

sys.path.insert(0, "/opt/trn_rl_repo")

import numpy as np
import ml_dtypes

import concourse.bass as bass
import concourse.bacc as bacc
import concourse.tile as tile
import concourse.mybir as mybir
from concourse.bass_utils import run_bass_kernel_spmd

F32 = mybir.dt.float32
BF16 = mybir.dt.bfloat16
I16 = mybir.dt.int16
AF = mybir.ActivationFunctionType
ALU = mybir.AluOpType
BF16NP = ml_dtypes.bfloat16

N, T, D, H, DH, DFF = 10000, 12, 64, 4, 16, 128
NCORES = 8
CHUNK = N // NCORES          # 1250
WIN = 128                    # dst-window size (nodes)
NW = (CHUNK + WIN - 1) // WIN  # 10 windows; last has 98 nodes
EPS = 1e-5
NEG_SLOPE = 0.2
ZROW = 896                   # padded zpack row (bf16 elems): 12*68 data + 80 pad
NBLK = (N + 127) // 128      # 79 phase-1 blocks (last = 16 nodes)
NPAIR = T // 2               # 6 paired (2-timestep) transposes per block
SUP = 4                      # phase-1 super-block (batched DMA + rsqrt math)
PREPD = 2                    # gather prepare_only lookahead depth (= zg bufs)

# rsqrt(v + EPS) = quadratic fit + one Newton step (vector engine only).
_BN1_RANGE = (0.55, 1.6)
_BN2_RANGE = (0.55, 3.2)


def _rsqrt_coeffs(lo, hi):
    v = np.linspace(lo, hi, 4001)
    c = np.polyfit(v, 1.0 / np.sqrt(v + EPS), 2)
    return [float(x) for x in c]


def _win_nodes(w):
    return min(WIN, CHUNK - w * WIN)


def _prep_core_rel(src, dst, lo, B):
    """Edge lists for one (core, relation): sorted by dst, windowed, padded
    to B blocks of 128 edges per window. Returns (src_flat[NW*B*128], S, ST)
    with S[w, e_in_block, blk*128 + dst_local] and its per-block transpose
    ST[w, dst_local, blk*128 + e_in_block]."""
    hi = lo + CHUNK
    sel = (dst >= lo) & (dst < hi)
    es = src[sel].astype(np.int64)
    ed = (dst[sel] - lo).astype(np.int64)
    order = np.argsort(ed, kind="stable")
    es, ed = es[order], ed[order]
    L = NW * B * 128
    src_arr = np.zeros(L, np.int64)
    S = np.zeros((NW, 128, B * 128), BF16NP)
    ST = np.zeros((NW, 128, B * 128), BF16NP)
    wstart = np.searchsorted(ed, np.arange(NW) * WIN)
    wend = np.searchsorted(ed, np.arange(1, NW + 1) * WIN)
    for w in range(NW):
        seg_src = es[wstart[w]:wend[w]]
        seg_dst = ed[wstart[w]:wend[w]] - w * WIN
        cnt = len(seg_src)
        assert cnt <= B * 128
        base = w * B * 128
        src_arr[base:base + cnt] = seg_src
        i = np.arange(cnt)
        S[w, i % 128, (i // 128) * 128 + seg_dst] = 1.0
        ST[w, seg_dst, (i // 128) * 128 + (i % 128)] = 1.0
    return src_arr, S, ST


def _max_blocks(src, dst):
    best = 0
    for m in range(NCORES):
        lo = m * CHUNK
        sel = (dst >= lo) & (dst < lo + CHUNK)
        ed = dst[sel] - lo
        cnt = np.bincount(ed // WIN, minlength=NW)
        best = max(best, int(np.max((cnt + 127) // 128)))
    return best


def _emit_rsqrt(nc, pool, vcol, P, ncols, coeffs, tag):
    """rs = rsqrt(vcol + EPS) via quadratic + 1 Newton step, on vector."""
    Q2, Q1, Q0 = coeffs
    rs = pool.tile([128, ncols], F32, tag=f"rs{tag}")
    nc.vector.tensor_scalar(rs[:P], vcol, Q2, Q1, ALU.mult, ALU.add)
    nc.vector.tensor_mul(rs[:P], rs[:P], vcol)
    nc.vector.tensor_scalar(rs[:P], rs[:P], Q0, None, ALU.add)
    vep = pool.tile([128, ncols], F32, tag=f"vep{tag}")
    nc.vector.tensor_scalar(vep[:P], vcol, EPS, None, ALU.add)
    t_ = pool.tile([128, ncols], F32, tag=f"tn{tag}")
    nc.vector.tensor_mul(t_[:P], rs[:P], rs[:P])
    nc.vector.tensor_mul(t_[:P], t_[:P], vep[:P])
    nc.vector.tensor_scalar(t_[:P], t_[:P], -0.5, 1.5, ALU.mult, ALU.add)
    nc.vector.tensor_mul(rs[:P], rs[:P], t_[:P])
    return rs


def _build_program(B, phases=3):
    nc = bacc.Bacc("TRN2", target_bir_lowering=False, debug=False,
                   num_devices=NCORES)
    BL = B * 128               # padded edges per (window, rel)
    BL2 = 2 * BL
    W16 = BL2 // 16            # idx cols per window (both rels)
    L16 = NW * W16
    RC1 = _rsqrt_coeffs(*_BN1_RANGE)
    RC2 = _rsqrt_coeffs(*_BN2_RANGE)

    # ---- DRAM tensors ----
    x_full = nc.dram_tensor("x_full", [N, T * D], F32, kind="ExternalInput")
    xc = nc.dram_tensor("xc", [CHUNK, T * D], F32, kind="ExternalInput")
    bn1_gb = nc.dram_tensor("bn1_gb", [N, 2], F32, kind="ExternalInput")
    bn1_gbc = nc.dram_tensor("bn1_gbc", [CHUNK, 2], F32, kind="ExternalInput")
    bn2_gb = nc.dram_tensor("bn2_gb", [CHUNK, 2], F32, kind="ExternalInput")
    w_in, al_in, ar_in, s_in, st_in = [], [], [], [], []
    for r in (1, 2):
        w_in.append(nc.dram_tensor(f"W{r}", [D, H * DH], F32, kind="ExternalInput"))
        al_in.append(nc.dram_tensor(f"al{r}t", [D, H * DH], F32, kind="ExternalInput"))
        ar_in.append(nc.dram_tensor(f"ar{r}t", [D, H * DH], F32, kind="ExternalInput"))
        s_in.append(nc.dram_tensor(f"S{r}", [NW, 128, BL], BF16, kind="ExternalInput"))
        st_in.append(nc.dram_tensor(f"ST{r}", [NW, 128, BL], mybir.dt.float8e4, kind="ExternalInput"))
    si_in = nc.dram_tensor("srcidx", [128, L16], I16, kind="ExternalInput")
    ffw1_in = nc.dram_tensor("ffw1", [D, DFF], F32, kind="ExternalInput")
    ffb1_in = nc.dram_tensor("ffb1", [DFF, 1], F32, kind="ExternalInput")
    ffw2_in = nc.dram_tensor("ffw2", [DFF, D], F32, kind="ExternalInput")
    ffb2_in = nc.dram_tensor("ffb2", [D, 1], F32, kind="ExternalInput")
    ident_in = nc.dram_tensor("ident", [128, 128], BF16, kind="ExternalInput")
    out_d = nc.dram_tensor("OUT", [CHUNK, T * D], F32, kind="ExternalOutput")

    # interleaved: row 2*node + rel
    zpackB = nc.dram_tensor("zpackB", [2 * N, ZROW], BF16, kind="Internal")

    with tile.TileContext(nc) as tc:
        with (
            tc.tile_pool(name="const", bufs=1) as cpool,
            tc.tile_pool(name="zg", bufs=PREPD) as zgp,
        ):
            # ---- Phase 0 ----
            ident = cpool.tile([128, 128], BF16)
            nc.sync.dma_start(ident[:], ident_in[:])
            wpair = cpool.tile([128, 2, 2, 68], BF16)
            nc.vector.memset(wpair[:], 0.0)
            wer = cpool.tile([128, 2, 2, H], BF16)
            nc.vector.memset(wer[:], 0.0)
            for r in range(2):
                wf = cpool.tile([D, H * DH], F32, tag="wf")
                nc.sync.dma_start(wf[:], w_in[r][:])
                for par in range(2):
                    nc.vector.tensor_copy(
                        wpair[par * D:(par + 1) * D, r, par, 0:64], wf[:])
                for which, t_in in (("al", al_in[r]), ("ar", ar_in[r])):
                    alt = cpool.tile([D, H * DH], F32, tag="alt")
                    nc.sync.dma_start(alt[:], t_in[:])
                    prod = cpool.tile([D, H * DH], F32, tag="prod")
                    nc.vector.tensor_mul(prod[:], wf[:], alt[:])
                    red = cpool.tile([D, H], F32, tag="red")
                    nc.vector.tensor_reduce(
                        red[:].unsqueeze(2),
                        prod[:].rearrange("p (h k) -> p h k", k=DH),
                        mybir.AxisListType.X, ALU.add)
                    for par in range(2):
                        if which == "al":
                            nc.vector.tensor_copy(
                                wpair[par * D:(par + 1) * D, r, par, 64:68],
                                red[:])
                        else:
                            nc.vector.tensor_copy(
                                wer[par * D:(par + 1) * D, par, r, :], red[:])
            ffw1 = cpool.tile([128, DFF], BF16)
            t1 = cpool.tile([D, DFF], F32, tag="t1")
            nc.sync.dma_start(t1[:], ffw1_in[:])
            nc.vector.tensor_copy(ffw1[0:D, :], t1[:])
            nc.sync.dma_start(ffw1[64:128, :], ffw1[0:64, :])
            ffw2 = cpool.tile([DFF, D], BF16)
            t2 = cpool.tile([DFF, D], F32, tag="t2")
            nc.sync.dma_start(t2[:], ffw2_in[:])
            nc.vector.tensor_copy(ffw2[:], t2[:])
            ffb1 = cpool.tile([DFF, 1], F32)
            nc.sync.dma_start(ffb1[:], ffb1_in[:])
            ffb2r = cpool.tile([128, 1], F32)
            nc.sync.dma_start(ffb2r[0:64, :], ffb2_in[:])
            nc.sync.dma_start(ffb2r[64:128, :], ffb2_in[:])
            si_all = cpool.tile([128, L16], I16)
            nc.sync.dma_start(si_all[:], si_in[:])

            # one-ahead gather issue; gpsimd runs ONLY gathers so the
            # next window's descriptor-gen overlaps this window's compute
            zgs = {}

            def issue_gather(w):
                zg = zgp.tile([128, 2 * B, ZROW], BF16, tag="zg")
                nc.gpsimd.dma_gather(
                    zg[:], zpackB[:],
                    si_all[:, w * W16:(w + 1) * W16],
                    BL2, BL2, ZROW, single_packet=False)
                zgs[w] = zg

            # ---- Phase 1: BN1 + projections for all N nodes ----
            with (
                tc.tile_pool(name="p1x", bufs=2) as p1x,
                tc.tile_pool(name="p1z", bufs=3) as p1z,
                tc.tile_pool(name="p1h", bufs=3) as p1h,
                tc.tile_pool(name="p1s", bufs=3) as p1s,
                tc.tile_pool(name="p1tp", bufs=2, space="PSUM") as p1tp,
                tc.tile_pool(name="p1zp", bufs=1, space="PSUM") as p1zp,
            ):
                def stage_a(sb):
                    # batched x/gb DMA + stats + rsqrt poly for one super
                    nsb = min(SUP, NBLK - sb)
                    n0 = sb * 128
                    nn = min(SUP * 128, N - n0)
                    full = (nn == nsb * 128)
                    xt4 = p1x.tile([128, SUP, T * D], F32, tag="xt4")
                    gbt = p1s.tile([128, SUP, 2], F32, tag="gbt")
                    if full:
                        nc.sync.dma_start(
                            xt4[:, 0:nsb, :],
                            x_full[n0:n0 + nn].rearrange(
                                "(j p) c -> p j c", p=128))
                        nc.sync.dma_start(
                            gbt[:, 0:nsb, :],
                            bn1_gb[n0:n0 + nn].rearrange(
                                "(j p) c -> p j c", p=128))
                    mvt = p1s.tile([128, SUP, 2], F32, tag="mvt")
                    for j in range(nsb):
                        nb = min(128, N - (sb + j) * 128)
                        if not full:
                            nc.sync.dma_start(
                                xt4[:nb, j, :],
                                x_full[(sb + j) * 128:(sb + j) * 128 + nb])
                            nc.sync.dma_start(
                                gbt[:nb, j, :],
                                bn1_gb[(sb + j) * 128:(sb + j) * 128 + nb])
                        st6 = p1s.tile([128, 2, 6], F32, tag="st6")
                        nc.vector.bn_stats(st6[:nb, 0, :], xt4[:nb, j, 0:384])
                        nc.vector.bn_stats(st6[:nb, 1, :],
                                           xt4[:nb, j, 384:768])
                        nc.vector.bn_aggr(mvt[:nb, j, :], st6[:nb])
                    rs = _emit_rsqrt(nc, p1s, mvt[:, 0:nsb, 1], 128, nsb,
                                     RC1, "p1")
                    ab = p1s.tile([128, SUP, 2], F32, tag="ab")
                    nc.vector.tensor_mul(ab[:, 0:nsb, 0], gbt[:, 0:nsb, 0],
                                         rs[:, 0:nsb])
                    nc.vector.tensor_mul(ab[:, 0:nsb, 1], ab[:, 0:nsb, 0],
                                         mvt[:, 0:nsb, 0])
                    nc.vector.tensor_sub(ab[:, 0:nsb, 1], gbt[:, 0:nsb, 1],
                                         ab[:, 0:nsb, 1])
                    return xt4, ab, nsb

                for sb in range(0, NBLK, SUP):
                    xt4, ab, nsb = stage_a(sb)
                    for j in range(nsb):
                        blk = sb + j
                        nb = min(128, N - blk * 128)
                        h = p1h.tile([128, T * D], BF16, tag="h")
                        nc.scalar.activation(h[:nb], xt4[:nb, j, :],
                                             AF.Identity,
                                             bias=ab[:nb, j, 1:2],
                                             scale=ab[:nb, j, 0:1])
                        tp = p1tp.tile([128, NPAIR, 128], BF16, tag="tp")
                        for p in range(NPAIR):
                            nc.tensor.transpose(
                                tp[:, p, 0:nb], h[:nb, p * 128:(p + 1) * 128],
                                ident[:nb, :nb])
                        ht = p1h.tile([128, NPAIR, 128], BF16, tag="ht")
                        if blk % 2 == 0:
                            nc.scalar.activation(ht[:, :, 0:nb],
                                                 tp[:, :, 0:nb], AF.Copy)
                        else:
                            nc.vector.tensor_copy(ht[:, :, 0:nb],
                                                  tp[:, :, 0:nb])
                        # one PSUM bank (512 f32) per pair: matmul outputs
                        # must never cross a 2KB PSUM bank boundary
                        zp = p1zp.tile([128, NPAIR, 512], F32, tag="zp")
                        for p in range(NPAIR):
                            nc.tensor.matmul(
                                zp[0:nb, p, 0:272], ht[:, p, 0:nb],
                                wpair[:].rearrange("p a b c -> p (a b c)"),
                                start=True, stop=True)
                        zel2 = p1z.tile([128, 2, T * 68], BF16, tag="zel2")
                        for r in range(2):
                            src_ap = zp[:nb, :, 0:272].rearrange(
                                "p q (r par c) -> p q r par c",
                                r=2, c=68)[:, :, r, :, :]
                            dst_ap = zel2[:nb, r, :].rearrange(
                                "p (q par c) -> p q par c", q=NPAIR, c=68)
                            if (blk + r) % 2 == 0:
                                nc.scalar.activation(dst_ap, src_ap, AF.Copy)
                            else:
                                nc.vector.tensor_copy(dst_ap, src_ap)
                        nc.sync.dma_start(
                            zpackB[2 * blk * 128:2 * blk * 128 + 2 * nb,
                                   0:T * 68],
                            zel2[:nb].rearrange("p r c -> p (r c)"))

            # ---- Phase 2: fused gather/attention/segment-sum/BN2/FFN ----
            with (
                tc.tile_pool(name="x2p", bufs=2) as x2p,
                tc.tile_pool(name="abp", bufs=2) as abp,
                tc.tile_pool(name="sp", bufs=2) as spp,
                tc.tile_pool(name="msg", bufs=2) as msgp,
                tc.tile_pool(name="p2s", bufs=4) as p2s,
                tc.tile_pool(name="p2t", bufs=2) as p2t,
                tc.tile_pool(name="pp", bufs=1, space="PSUM") as pp,
            ):
                pre = {}

                def prefetch(w):
                    nw = _win_nodes(w)
                    xcw = p2t.tile([128, T * D], F32, tag="xcw")
                    nc.sync.dma_start(xcw[:nw], xc[w * WIN:w * WIN + nw])
                    gb = p2s.tile([128, 2], F32, tag="gbw", bufs=2)
                    nc.sync.dma_start(gb[:nw], bn1_gbc[w * WIN:w * WIN + nw])
                    gb2 = p2s.tile([128, 2], F32, tag="gb2", bufs=2)
                    nc.sync.dma_start(gb2[:nw], bn2_gb[w * WIN:w * WIN + nw])
                    ss = []
                    for r in range(2):
                        ssb = spp.tile([128, BL], BF16, tag=f"ssb{r}")
                        nc.sync.dma_start(ssb[:], s_in[r][w])
                        stb = spp.tile([128, BL], mybir.dt.float8e4,
                                       tag=f"stb{r}", bufs=1)
                        nc.sync.dma_start(stb[:], st_in[r][w])
                        ss.append((ssb, stb))
                    pre[w] = (xcw, gb, gb2, ss)

                erws = {}

                def er_chain(w):
                    # one window ahead: only needs the prefetched x rows
                    nw = _win_nodes(w)
                    xcw, gb, gb2, ss = pre[w]
                    st6 = p2s.tile([128, 2, 6], F32, tag="st6w")
                    nc.vector.bn_stats(st6[:nw, 0, :], xcw[:nw, 0:384])
                    nc.vector.bn_stats(st6[:nw, 1, :], xcw[:nw, 384:768])
                    mv = p2s.tile([128, 2], F32, tag="mvw")
                    nc.vector.bn_aggr(mv[:nw], st6[:nw])
                    rsw = _emit_rsqrt(nc, p2s, mv[:nw, 1:2], nw, 1, RC1, "w")
                    a1 = p2s.tile([128, 2], F32, tag="a1w")
                    nc.vector.tensor_mul(a1[:nw, 0:1], gb[:nw, 0:1], rsw[:nw])
                    nc.vector.tensor_mul(a1[:nw, 1:2], a1[:nw, 0:1],
                                         mv[:nw, 0:1])
                    nc.vector.tensor_sub(a1[:nw, 1:2], gb[:nw, 1:2],
                                         a1[:nw, 1:2])
                    hw_ = p2t.tile([128, T * D], BF16, tag="hw")
                    nc.scalar.activation(hw_[:nw], xcw[:nw], AF.Identity,
                                         bias=a1[:nw, 1:2], scale=a1[:nw, 0:1])
                    tpw = pp.tile([128, NPAIR, 128], BF16, tag="tpS")
                    for p in range(NPAIR):
                        nc.tensor.transpose(
                            tpw[:, p, 0:nw], hw_[:nw, p * 128:(p + 1) * 128],
                            ident[:nw, :nw])
                    htw = p2t.tile([128, NPAIR, 128], BF16, tag="htw")
                    nc.vector.tensor_copy(htw[:, :, 0:nw], tpw[:, :, 0:nw])
                    erps = pp.tile([128, NPAIR, 2 * 2 * H], F32, tag="erps")
                    for p in range(NPAIR):
                        nc.tensor.matmul(
                            erps[0:nw, p, :], htw[:, p, 0:nw],
                            wer[:].rearrange("p a b c -> p (a b c)"),
                            start=True, stop=True)
                    erw = p2s.tile([128, 2, T, H], mybir.dt.float8e4,
                                   tag="erw", bufs=2)
                    if nw < 128:
                        nc.vector.memset(erw[:], 0.0)
                    nc.vector.tensor_copy(
                        erw[:nw].rearrange("p r (q par) h -> p q par r h",
                                           par=2),
                        erps[:nw].rearrange("p q (par r h) -> p q par r h",
                                            par=2, r=2))
                    erws[w] = erw

                prefetch(0)
                issue_gather(0)
                er_chain(0)
                for w in range(NW):
                    nw = _win_nodes(w)
                    if w + 1 < NW:
                        issue_gather(w + 1)
                        prefetch(w + 1)
                        er_chain(w + 1)
                    zg = zgs.pop(w)
                    xcw, gb, gb2, ss = pre.pop(w)
                    erw = erws.pop(w)
                    msum = []
                    for r in range(2):
                        ssb, stb = ss[r]
                        lk = p2s.tile([128, B, T * H], BF16, tag="lk")
                        HB = (B + 1) // 2
                        for half in range(2):
                            b0 = half * HB
                            b1 = min(b0 + HB, B)
                            if b0 >= b1:
                                continue
                            ebc = pp.tile([128, HB, T * H], F32, tag="ebc")
                            for b in range(b0, b1):
                                nc.tensor.matmul(
                                    ebc[:, b - b0, :],
                                    stb[:, b * 128:(b + 1) * 128],
                                    erw[:, r, :, :].rearrange(
                                        "p q h -> p (q h)"),
                                    start=True, stop=True)
                            el_ap = zg[:, r * B + b0:r * B + b1, 0:T * 68] \
                                .rearrange("p b (t c) -> p b t c",
                                           c=68)[:, :, :, 64:68]
                            nc.vector.tensor_add(
                                lk[:, b0:b1].rearrange(
                                    "p b (t h) -> p b t h", h=H),
                                el_ap,
                                ebc[:, 0:b1 - b0].rearrange(
                                    "p b (t h) -> p b t h", h=H))
                        nc.vector.scalar_tensor_tensor(
                            lk[:], lk[:], NEG_SLOPE, lk[:], ALU.mult, ALU.max)
                        msgb = msgp.tile([128, B, 816], BF16, tag="msg")
                        nc.scalar.activation(
                            msgb[:, :, 0:768].rearrange(
                                "p b (t h k) -> p b t h k", h=H, k=DH),
                            lk[:].rearrange("p b (t h) -> p b t h", h=H)
                            .unsqueeze(4).broadcast_to((128, B, T, H, DH)),
                            AF.Exp)
                        nc.scalar.activation(msgb[:, :, 768:816], lk[:],
                                             AF.Exp)
                        zap = zg[:, r * B:(r + 1) * B, 0:T * 68].rearrange(
                            "p b (t c) -> p b t c", c=68)[:, :, :, 0:64]
                        mz = msgb[:, :, 0:768].rearrange(
                            "p b (t c) -> p b t c", c=64)
                        nc.vector.tensor_mul(mz, mz, zap)
                        ms = pp.tile([128, 816], F32, tag="big", bufs=2)
                        for b in range(B):
                            lhsT = ssb[:, b * 128:(b + 1) * 128]
                            nc.tensor.matmul(ms[:, 0:512], lhsT,
                                             msgb[:, b, 0:512],
                                             start=(b == 0), stop=(b == B - 1))
                            nc.tensor.matmul(ms[:, 512:816], lhsT,
                                             msgb[:, b, 512:816],
                                             start=(b == 0), stop=(b == B - 1))
                        msum.append(ms)
                    # epilogue: m = msgsum/denom; x2 = bf16(x + m1 + m2)
                    x2w = x2p.tile([128, T * D], BF16, tag="x2")
                    mtmp = p2t.tile([128, T * D], F32, tag="mtmp")
                    for r in range(2):
                        rec = p2s.tile([128, T * H], F32, tag="rec")
                        nc.vector.tensor_scalar_max(
                            rec[:nw], msum[r][:nw, 768:816], 1e-16)
                        nc.vector.reciprocal(rec[:nw], rec[:nw])
                        rb = rec[:nw].rearrange(
                            "p (t h) -> p t h", h=H).unsqueeze(3) \
                            .broadcast_to((nw, T, H, DH))
                        dst = (mtmp if r == 0 else x2w)
                        nc.vector.tensor_mul(
                            dst[:nw].rearrange(
                                "p (t h k) -> p t h k", h=H, k=DH),
                            msum[r][:nw, 0:768].rearrange(
                                "p (t h k) -> p t h k", h=H, k=DH), rb)
                    nc.vector.tensor_add(mtmp[:nw], mtmp[:nw], xcw[:nw])
                    nc.vector.tensor_add(x2w[:nw], x2w[:nw], mtmp[:nw])
                    # BN2 stats + a2/b2
                    st6b = p2s.tile([128, 2, 6], F32, tag="st6b")
                    nc.vector.bn_stats(st6b[:nw, 0, :], x2w[:nw, 0:384])
                    nc.vector.bn_stats(st6b[:nw, 1, :], x2w[:nw, 384:768])
                    mvb = p2s.tile([128, 2], F32, tag="mvb")
                    nc.vector.bn_aggr(mvb[:nw], st6b[:nw])
                    rs2 = _emit_rsqrt(nc, p2s, mvb[:nw, 1:2], nw, 1, RC2,
                                      "b2")
                    ab2 = abp.tile([128, 2], F32, tag="ab2")
                    nc.vector.tensor_mul(ab2[:nw, 0:1], gb2[:nw, 0:1],
                                         rs2[:nw])
                    nc.vector.tensor_mul(ab2[:nw, 1:2], ab2[:nw, 0:1],
                                         mvb[:nw, 0:1])
                    nc.vector.tensor_sub(ab2[:nw, 1:2], gb2[:nw, 1:2],
                                         ab2[:nw, 1:2])
                    if phases < 3:
                        xo = p2t.tile([128, T * D], F32, tag="mtmp")
                        nc.vector.tensor_copy(xo[:nw], x2w[:nw])
                        nc.sync.dma_start(out_d[w * WIN:w * WIN + nw],
                                          xo[:nw])
                        continue
                    # ---- FFN (interleaved): BN2 apply + 2 layers + res ----
                    h2 = p2t.tile([128, T * D], BF16, tag="hw")
                    nc.scalar.activation(h2[:nw], x2w[:nw], AF.Identity,
                                         bias=ab2[:nw, 1:2],
                                         scale=ab2[:nw, 0:1])
                    h2t = p2t.tile([64, T, 128], BF16, tag="h2t")
                    for half in range(2):
                        tp = pp.tile([64, NPAIR, 128], BF16, tag="tp3")
                        for j in range(NPAIR):
                            t = half * NPAIR + j
                            nc.tensor.transpose(
                                tp[:, j, 0:nw], h2[:nw, t * 64:(t + 1) * 64],
                                ident[:nw, :nw])
                        nc.vector.tensor_copy(
                            h2t[:, half * NPAIR:(half + 1) * NPAIR, 0:nw],
                            tp[:, :, 0:nw])
                    if nw < 128:
                        nc.vector.memset(h2t[:, :, nw:128], 0.0)
                    dd = pp.tile([128, T, 64], BF16, tag="tpS")
                    fft = p2t.tile([64, T, 128], BF16, tag="fft")
                    for k in range(3):
                        big = pp.tile([128, 816], F32, tag="big", bufs=2)
                        f1 = big[:, 0:512]
                        rhs = h2t[:, 4 * k:4 * k + 4, :]
                        nc.tensor.matmul(f1, ffw1[0:64, :],
                                         rhs.rearrange("p a b -> p (a b)"),
                                         start=True, stop=True)
                        g1 = p2t.tile([128, 512], BF16, tag="g1")
                        nc.scalar.activation(g1[:], f1, AF.Gelu, bias=ffb1[:])
                        for half in range(2):
                            f2 = big[0:64, 512:768]
                            nc.tensor.matmul(
                                f2, ffw2[:],
                                g1[:, half * 256:(half + 1) * 256],
                                start=True, stop=True)
                            nc.scalar.activation(
                                fft[:, 4 * k + 2 * half:
                                    4 * k + 2 * half + 2, :]
                                .rearrange("p a b -> p (a b)"),
                                f2, AF.Identity, bias=ffb2r[0:64, :])
                    for t in range(T):
                        nc.tensor.transpose(
                            dd[0:nw, t, :], fft[:, t, 0:nw],
                            ident[0:64, 0:64])
                    ot = p2t.tile([128, T * D], F32, tag="mtmp")
                    nc.vector.tensor_add(
                        ot[:nw], dd[:nw].rearrange("p a b -> p (a b)"),
                        x2w[:nw])
                    nc.sync.dma_start(out_d[w * WIN:w * WIN + nw], ot[:nw])

    nc.compile()
    return nc


_CACHE = {}
_PHASES = 3
_TRACE = False
_TRACE_DIR = None
_LAST_EXEC_NS = None


def _host_prep(inputs):
    x = np.asarray(inputs["x"], np.float32)
    xf = np.ascontiguousarray(x.reshape(N, T * D))
    B = 0
    for r in (1, 2):
        B = max(B, _max_blocks(np.asarray(inputs[f"src{r}"]),
                               np.asarray(inputs[f"dst{r}"])))

    bn1_gb = np.ascontiguousarray(
        np.stack([np.asarray(inputs["bn1_g"], np.float32),
                  np.asarray(inputs["bn1_b"], np.float32)], axis=1))
    bn2_gb_full = np.ascontiguousarray(
        np.stack([np.asarray(inputs["bn2_g"], np.float32),
                  np.asarray(inputs["bn2_b"], np.float32)], axis=1))
    common = {
        "x_full": xf,
        "bn1_gb": bn1_gb,
        "ffw1": np.ascontiguousarray(np.asarray(inputs["ff_w1"], np.float32)),
        "ffb1": np.ascontiguousarray(
            np.asarray(inputs["ff_b1"], np.float32).reshape(DFF, 1)),
        "ffw2": np.ascontiguousarray(np.asarray(inputs["ff_w2"], np.float32)),
        "ffb2": np.ascontiguousarray(
            np.asarray(inputs["ff_b2"], np.float32).reshape(D, 1)),
        "ident": np.eye(128, dtype=BF16NP),
    }
    for r in (1, 2):
        W = np.asarray(inputs[f"W{r}"], np.float32).reshape(D, H * DH)
        al = np.asarray(inputs[f"al{r}"], np.float32).reshape(-1)
        ar = np.asarray(inputs[f"ar{r}"], np.float32).reshape(-1)
        common[f"W{r}"] = np.ascontiguousarray(W)
        common[f"al{r}t"] = np.ascontiguousarray(np.tile(al[None, :], (D, 1)))
        common[f"ar{r}t"] = np.ascontiguousarray(np.tile(ar[None, :], (D, 1)))

    BL = B * 128
    in_maps = []
    for m in range(NCORES):
        lo = m * CHUNK
        im = dict(common)
        im["xc"] = np.ascontiguousarray(xf[lo:lo + CHUNK])
        im["bn1_gbc"] = np.ascontiguousarray(bn1_gb[lo:lo + CHUNK])
        im["bn2_gb"] = np.ascontiguousarray(bn2_gb_full[lo:lo + CHUNK])
        srcs = []
        for r in (1, 2):
            src_flat, S, ST = _prep_core_rel(
                np.asarray(inputs[f"src{r}"]), np.asarray(inputs[f"dst{r}"]),
                lo, B)
            im[f"S{r}"] = S
            im[f"ST{r}"] = ST.astype(ml_dtypes.float8_e4m3)
            srcs.append(2 * src_flat + (r - 1))  # interleaved zpackB rows
        idx = np.empty((128, NW * (2 * BL) // 16), np.int16)
        for w in range(NW):
            for r in range(2):
                seg = srcs[r][w * BL:(w + 1) * BL].astype(np.int16)
                col0 = (w * 2 + r) * (BL // 16)
                idx[:, col0:col0 + BL // 16] = np.tile(
                    seg.reshape(-1, 16).T, (8, 1))
        im["srcidx"] = np.ascontiguousarray(idx)
        in_maps.append(im)
    return B, in_maps


def kernel(**inputs):
    B, in_maps = _host_prep(inputs)
    key = (B, _PHASES)
    if key not in _CACHE:
        _CACHE[key] = _build_program(B, _PHASES)
    nc = _CACHE[key]
    global _LAST_EXEC_NS
    tmpdir = None
    if _TRACE and _TRACE_DIR:
        import os, shutil
        shutil.rmtree(_TRACE_DIR, ignore_errors=True)
        os.makedirs(_TRACE_DIR, exist_ok=True)
        tmpdir = _TRACE_DIR
    res = run_bass_kernel_spmd(nc, in_maps, core_ids=list(range(NCORES)),
                               trace=_TRACE, tmpdir=tmpdir)
    _LAST_EXEC_NS = res.exec_time_ns
    out = np.concatenate([res.results[m]["OUT"] for m in range(NCORES)],
                         axis=0)
    return out.reshape(N, T, D).astype(np.float32)
